# revision 3
# baseline (speedup 1.0000x reference)
"""Trainium2 Bass kernel for nn_BatchLossFunction_38534446579748.

Loss:  cos = <pt[b,p,:], ot[b,:]> / (max(||pt||,eps) * max(||ot||,eps))
       v   = sigmoid(1 - cos);  gtv = gt.reshape(B,196)/255
       loss = sum(-log(1 - |v - gtv|) * (gtv*GAMMA + 1)) / B

Strategy (pure data parallel over batch, 8 cores x 256 batches):
  Dual DMA streams per core (queue-level parallelism beats the ~355 GB/s
  single-queue ceiling; the two queues round-robin ~50:50, so bytes are
  split evenly):
    - q0 (SWDGE, gpsimd): cast-on-DMA f32->bf16 chunks -> DVE fold path
    - q1 (HWDGE, sync):   plain f32 chunks             -> STT/ACT path
  Compute split (measured per-op costs):
    - DVE fold-dot per 14-patch bf16 chunk: z = x * bcast(ot) (TT bf16
      2x_1p) + 5 binary folds + segmented tensor_reduce = 891ns/patch,
      vs STT 1x at 1032ns/patch for f32 patches.
    - ACT Square+accum (PSUM out/accum: 904+215ns) takes most sumsq; three
      bf16 chunks' sumsq run as DVE fold-sq batches to balance engines.
  Pipelining: each cast-DMA is issued one unit ahead of its compute
  (crossing the group boundary) so folds never head-of-line block; g0's
  epilogue stages ride the early g1 units; g1 ends with a pure-DVE chunk
  so only g1's short epilogue chain remains after the last DMA.
"""

import os
import sys

import numpy as np

if "/opt/trn_rl_repo" not in sys.path:
    sys.path.insert(0, "/opt/trn_rl_repo")

from contextlib import ExitStack

import concourse.bacc as bacc
import concourse.tile as tile
from concourse import mybir
from concourse.bass_utils import run_bass_kernel_spmd

N_CORES = 8
B, P, D = 2048, 197, 768
NP = P - 1          # 196 usable patch tokens
BS = B // N_CORES   # 256 batches per core
PART = 128          # SBUF partitions
G = BS // PART      # 2 groups of 128 batches per core
GAMMA = 3.4
EPS = 1e-8
CH = 14

F32 = mybir.dt.float32
BF16 = mybir.dt.bfloat16
ALU = mybir.AluOpType
ACTF = mybir.ActivationFunctionType

_CACHE = {}


def _build():
    nc = bacc.Bacc("TRN2", target_bir_lowering=False, debug=False)

    pt = nc.dram_tensor("patch_tokens", [BS, P, D], F32, kind="ExternalInput")
    ot = nc.dram_tensor("out_text", [BS, D], F32, kind="ExternalInput")
    gt = nc.dram_tensor("gt", [BS, 14, 14], F32, kind="ExternalInput")
    out = nc.dram_tensor("loss_parts", [PART, G], F32, kind="ExternalOutput")

    pt_ap = pt.ap()
    ot_ap = ot.ap()
    gt_ap = gt.ap().rearrange("b h w -> b (h w)")
    out_ap = out.ap()

    with ExitStack() as ctx:
        tc = ctx.enter_context(tile.TileContext(nc))
        xc_pool = ctx.enter_context(tc.tile_pool(name="xc", bufs=3))   # bf16 chunks
        xf_pool = ctx.enter_context(tc.tile_pool(name="xf", bufs=4))   # f32 chunks (7-patch)
        persist = ctx.enter_context(tc.tile_pool(name="persist", bufs=1))
        psum = ctx.enter_context(tc.tile_pool(name="psum", bufs=1, space="PSUM"))

        trashP = psum.tile([PART, D], F32, tag="trashP")       # ACT main out
        trash_v = persist.tile([PART, D], BF16, tag="trash_v")  # DVE STT out (bf16 to save SBUF)
        loss = persist.tile([PART, G], F32, tag="loss")

        z = persist.tile([PART, CH, D], BF16, tag="z")
        fa = persist.tile([PART, CH, 384], BF16, tag="fa")

        # ---- prologue: out_text (f32 + bf16), ot sumsq; gt + W ----
        ots, ot16s, otsqs, gtts, ws = [], [], [], [], []
        for g in range(G):
            b0 = g * PART
            otile = persist.tile([PART, D], F32, tag=f"ot{g}")
            nc.sync.dma_start(out=otile, in_=ot_ap[b0 : b0 + PART, :])
            ot16 = persist.tile([PART, D], BF16, tag=f"ot16_{g}")
            nc.vector.tensor_copy(out=ot16, in_=otile)
            otsq = psum.tile([PART, 1], F32, tag=f"otsq{g}")
            nc.scalar.activation(
                out=trashP, in_=otile, func=ACTF.Square, accum_out=otsq
            )
            ots.append(otile)
            ot16s.append(ot16)
            otsqs.append(otsq)
        for g in range(G):
            b0 = g * PART
            gtt = persist.tile([PART, NP], F32, tag=f"gtt{g}", name=f"gtt{g}")
            nc.sync.dma_start(out=gtt, in_=gt_ap[b0 : b0 + PART, :])
            gtts.append(gtt)
            w = persist.tile([PART, NP], F32, tag=f"w{g}", name=f"w{g}")
            nc.scalar.activation(
                out=w, in_=gtt, func=ACTF.Copy, scale=-GAMMA / 255.0, bias=-1.0
            )
            ws.append(w)


        sss, ssvs, dts = [], [], []
        seglists = [[] for _ in range(G)]   # (lo, hi) ranges owned by ssv (DVE)
        for g in range(G):
            ss = psum.tile([PART, NP], F32, tag=f"ss{g}", name=f"ss{g}")      # ACT sumsq (PSUM)
            ssv = persist.tile([PART, NP], F32, tag=f"ssv{g}", name=f"ssv{g}")  # DVE fold sumsq
            dt_ = persist.tile([PART, NP], F32, tag=f"dt{g}", name=f"dt{g}")  # dots (SBUF)
            sss.append(ss)
            ssvs.append(ssv)
            dts.append(dt_)

        # ---------- stream emission helpers ----------
        def fold_chain(cnt, out_slice):
            zz = z[:, 0:cnt, :]
            ff = fa[:, 0:cnt, :]
            nc.vector.tensor_tensor(out=ff, in0=zz[:, :, 0:384], in1=zz[:, :, 384:768], op=ALU.add)
            nc.vector.tensor_tensor(out=zz[:, :, 0:192], in0=ff[:, :, 0:192], in1=ff[:, :, 192:384], op=ALU.add)
            nc.vector.tensor_tensor(out=ff[:, :, 0:96], in0=zz[:, :, 0:96], in1=zz[:, :, 96:192], op=ALU.add)
            nc.vector.tensor_tensor(out=zz[:, :, 0:48], in0=ff[:, :, 0:48], in1=ff[:, :, 48:96], op=ALU.add)
            nc.vector.tensor_tensor(out=ff[:, :, 0:24], in0=zz[:, :, 0:24], in1=zz[:, :, 24:48], op=ALU.add)
            nc.vector.tensor_reduce(
                out=out_slice, in_=ff[:, :, 0:24],
                axis=mybir.AxisListType.X, op=ALU.add,
            )

        def emit_c_dma(g, p0, cnt):
            b0 = g * PART
            x16 = xc_pool.tile([PART, CH, D], BF16, tag="x16", name="x16")
            nc.gpsimd.dma_start(
                out=x16[:, 0:cnt, :],
                in_=pt_ap[b0 : b0 + PART, 1 + p0 : 1 + p0 + cnt, :],
            )
            return x16

        def emit_c_compute(g, x16, p0, cnt, sq_dve_cnt):
            ot_b = ot16s[g].unsqueeze(1).broadcast_to([PART, cnt, D])
            nc.vector.tensor_tensor(out=z[:, 0:cnt, :], in0=x16[:, 0:cnt, :], in1=ot_b, op=ALU.mult)
            fold_chain(cnt, dts[g][:, p0 : p0 + cnt])
            k = sq_dve_cnt
            if k:
                nc.vector.tensor_tensor(out=z[:, 0:k, :], in0=x16[:, 0:k, :], in1=x16[:, 0:k, :], op=ALU.mult)
                fold_chain(k, ssvs[g][:, p0 : p0 + k])
                seglists[g].append((p0, p0 + k))
            for j in range(k, cnt):
                p = p0 + j
                nc.scalar.activation(
                    out=trashP, in_=x16[:, j, :], func=ACTF.Square,
                    accum_out=sss[g][:, p : p + 1],
                )

        def emit_f(g, p0, cnt):
            b0 = g * PART
            xf = xf_pool.tile([PART, 7, D], F32, tag="xf")
            nc.sync.dma_start(
                out=xf[:, 0:cnt, :],
                in_=pt_ap[b0 : b0 + PART, 1 + p0 : 1 + p0 + cnt, :],
            )
            for j in range(cnt):
                p = p0 + j
                nc.vector.scalar_tensor_tensor(
                    out=trash_v, in0=xf[:, j, :], scalar=1.0, in1=ots[g],
                    op0=ALU.mult, op1=ALU.mult,
                    accum_out=dts[g][:, p : p + 1],
                )
                nc.scalar.activation(
                    out=trashP, in_=xf[:, j, :], func=ACTF.Square,
                    accum_out=sss[g][:, p : p + 1],
                )

        # ---------- epilogue stage helpers ----------
        ptns = [persist.tile([PART, NP], F32, tag=f"ptn{g}", name=f"ptn{g}") for g in range(G)]
        otns = [persist.tile([PART, 1], F32, tag=f"otn{g}", name=f"otn{g}") for g in range(G)]
        dens = [persist.tile([PART, NP], F32, tag=f"den{g}", name=f"den{g}") for g in range(G)]
        coss = [persist.tile([PART, NP], F32, tag=f"cos{g}", name=f"cos{g}") for g in range(G)]
        omds = [persist.tile([PART, NP], F32, tag=f"omd{g}", name=f"omd{g}") for g in range(G)]
        rps = dens     # reciprocal computed in place
        vts = coss     # sigmoid computed in place
        lnts = omds    # ln computed in place

        def epi_sqrt(g):  # ACT (Sqrt set); sources split between ss (ACT) / ssv (DVE)
            segs = sorted(seglists[g])
            pos = 0
            for lo, hi in segs:
                if pos < lo:
                    nc.scalar.activation(out=ptns[g][:, pos:lo], in_=sss[g][:, pos:lo], func=ACTF.Sqrt)
                nc.scalar.activation(out=ptns[g][:, lo:hi], in_=ssvs[g][:, lo:hi], func=ACTF.Sqrt)
                pos = hi
            if pos < NP:
                nc.scalar.activation(out=ptns[g][:, pos:NP], in_=sss[g][:, pos:NP], func=ACTF.Sqrt)
            nc.scalar.activation(out=otns[g], in_=otsqs[g], func=ACTF.Sqrt)

        def epi_dve_cos(g):  # DVE
            nc.vector.tensor_scalar_max(out=ptns[g], in0=ptns[g], scalar1=EPS)
            nc.vector.tensor_scalar_max(out=otns[g], in0=otns[g], scalar1=EPS)
            nc.vector.tensor_scalar_mul(out=dens[g], in0=ptns[g], scalar1=otns[g])
            nc.vector.reciprocal(out=rps[g], in_=dens[g])
            nc.vector.tensor_mul(out=coss[g], in0=dts[g], in1=rps[g])

        def epi_sigmoid(g):  # ACT (Sigmoid set)
            nc.scalar.activation(
                out=vts[g], in_=coss[g], func=ACTF.Sigmoid, bias=1.0, scale=-1.0
            )

        def epi_omd(g):  # DVE
            nc.vector.scalar_tensor_tensor(
                out=omds[g], in0=gtts[g], scalar=1.0 / 255.0, in1=vts[g],
                op0=ALU.mult, op1=ALU.subtract,
            )

        def epi_ln(g):  # ACT (Ln set)
            nc.scalar.activation(out=lnts[g], in_=omds[g], func=ACTF.Ln, bias=1.0)

        def epi_loss(g):  # DVE
            nc.vector.scalar_tensor_tensor(
                out=trash_v[:, :NP], in0=lnts[g], scalar=1.0, in1=ws[g],
                op0=ALU.mult, op1=ALU.mult,
                accum_out=loss[:, g : g + 1],
            )

        # ---------- streams ----------
        # bf16 c-chunks per group: [7, 14, 14, 14, 14, 14, 14, 7] (98p);
        # f32 f-chunks: 14 x 7p (98p). The c-DMA for chunk i is issued one
        # unit ahead of its compute so the fold never head-of-line blocks.
        # fold-sq on selected c-chunks balances ACT; g1's final c-chunk is
        # pure-DVE (fold-dot + fold-sq) so ACT can start the epilogue.
        CSIZES = [7, 14, 14, 14, 14, 14, 14, 7]
        CP0 = [0, 7, 21, 35, 49, 63, 77, 91]
        SQ7 = {(0, 2): 7, (0, 5): 7, (1, 2): 7, (1, 5): 7}
        nch = len(CSIZES)
        seq = [(g, i) for g in range(G) for i in range(nch)]
        fis = [0 for _ in range(G)]
        pend = emit_c_dma(seq[0][0], CP0[seq[0][1]], CSIZES[seq[0][1]])
        for idx, (g, i) in enumerate(seq):
            # prefetch the next c-chunk's DMA (crosses the group boundary)
            if idx + 1 < len(seq):
                g2, i2 = seq[idx + 1]
                nxt = emit_c_dma(g2, CP0[i2], CSIZES[i2])
            else:
                nxt = None
            for _ in range(2):
                if fis[g] < 14:
                    emit_f(g, 98 + fis[g] * 7, 7)
                    fis[g] += 1
            sq = SQ7.get((g, i), 0)
            if (g, i) == (1, nch - 1):
                sq = CSIZES[i]          # pure-DVE tail chunk
            emit_c_compute(g, pend, CP0[i], CSIZES[i], sq)
            pend = nxt
            # g0's epilogue rides the early g1 units (fills the boundary
            # hole; only g1's short chain remains after the last DMA)
            if g == 1:
                if i == 0:
                    epi_sqrt(0)
                elif i == 1:
                    epi_dve_cos(0)
                elif i == 2:
                    epi_sigmoid(0)
                elif i == 3:
                    epi_omd(0)
                elif i == 4:
                    epi_ln(0)
                elif i == 5:
                    epi_loss(0)

        # ---------- epilogue: only group 1 remains ----------
        epi_sqrt(1)
        epi_dve_cos(1)
        epi_sigmoid(1)
        epi_omd(1)
        epi_ln(1)
        epi_loss(1)

        nc.sync.dma_start(out=out_ap, in_=loss)

    nc.compile()
    return nc


def _get_nc():
    if "nc" not in _CACHE:
        _CACHE["nc"] = _build()
    return _CACHE["nc"]


def _run(in_maps, **kwargs):
    return run_bass_kernel_spmd(_get_nc(), in_maps, core_ids=list(range(N_CORES)), **kwargs)


def _make_in_maps(patch_tokens, out_text, gt):
    patch_tokens = np.ascontiguousarray(np.asarray(patch_tokens, dtype=np.float32))
    out_text = np.ascontiguousarray(np.asarray(out_text, dtype=np.float32))
    gt = np.ascontiguousarray(np.asarray(gt, dtype=np.float32))
    in_maps = []
    for c in range(N_CORES):
        sl = slice(c * BS, (c + 1) * BS)
        in_maps.append(
            {
                "patch_tokens": patch_tokens[sl],
                "out_text": out_text[sl],
                "gt": gt[sl],
            }
        )
    return in_maps


def kernel(patch_tokens, out_text, gt):
    res = _run(_make_in_maps(patch_tokens, out_text, gt))
    total = np.float64(0.0)
    for r in res.results:
        total += r["loss_parts"].astype(np.float64).sum()
    return np.float32(total / B)
